# revision 3
# baseline (speedup 1.0000x reference)
"""BitLinear (ternary-weight + 8-bit-activation quantized matmul) on 8 TRN2 cores.

Strategy: data-parallel over tokens. Each core gets 2048 of the 16384 tokens
plus the full weight matrix, computes the whole BitLinear forward for its
token shard on device, and the host concatenates the shards.

Math (must match the jax reference):
  w_scale = max(mean(|W|), 1e-6)                       (scalar)
  w_q     = clip(round(W / w_scale), -1, 1)            (ternary)
  a       = clip(max_i |x|, 1e-8, inf)                 (per token)
  x_q     = clip(round(x * 127 / a), -127, 127)        (8-bit ints)
  y       = (x_q @ w_q^T) * w_scale * a / 127

v3 design (roofline: 221us bf16 GEMM + ~50us W stream; v1 394 / v2 340):
  - w_scale is extremely sensitive (2e-4 rel deviation flips ternary weights
    near .5 boundaries -> 3e-2 err), so pass 1 must abs-sum the full fp32 W.
    W is fed TRANSPOSED (wT[k,out]) and is FULLY RESIDENT in fp32 (128KB of
    ~207KB/partition SBUF): zero re-read, W streams exactly once as 16 x 1MB
    DMAs all issued upfront with no pool waits -> HBM line rate.
  - pass-1 abs-sums alternate DVE reduce_sum (non-clobbering, even j) and
    ACT Abs->bf16 scratch with accum_out (odd j; bf16-rounded |W| sums are
    random-error ~1e-6 rel over 4.2M elements, harmless) so neither engine
    serializes the stream; w_scale lands ~2.5us after the last W tile.
  - the x side runs in BF16: x is cast f32->bf16 during the DMA itself
    (SWDGE/gpsimd ring, separate from the W stream's HWDGE ring). a_scale
    and x_q derive from the same bf16 x, so a cancels between quantize and
    rescale; measured 7.0e-3 total err (budget 2e-2). x_q flips from bf16
    x are per-token-random and wash out, unlike W-side flips.
  - rounding is the fp32 magic-number trick (+1.5*2^23, exact RNE; a bf16
    magic of +192 double-rounds and flips ~16 weights at the .5 boundary =
    up to ~1.8e-2 err, rejected). Quantize runs on [128,1024] halves
    through a shared 2-buf t1 pool. W: ACT magic -> DVE subtract into fp8
    (ints <= 8 exact in e4m3) -> full-row in-place fp8 clamp on GPSIMD
    (keeps DVE off the chase critical path). x: ACT magic -> DVE subtract
    to bf16, no clamp needed (|x*127/a| <= 127 by construction).
  - GEMM ramp: 8 PSUM cells (tiles 0-1 x 4 col-blocks) accumulate each b
    the moment its wq lands (PE chases the ACT-bound ~2.2us/b quantize
    stream at 1.7us/b), so the PE starts ~4us after w_scale.
  - steady state: x-prep pipelined two tiles ahead (xqT ring of 3), loads
    three ahead; per-iter budget PE 13.8us vs DVE ~7.2, ACT ~2.2.
  - y is stored bf16 (host upcasts); the last tile stores per-quarter to
    shorten the drain tail.
Dead ends (measured): sampled/bf16 w_scale (1.7-4e-2 err), sharded pass-1 +
AllReduce (~80us collective), fp8 DoubleRow x_q (exact hi/lo needs 2x
virtual MACs = breakeven minus overheads; single-pass fp8 approx 2.3e-2 >
budget), bf16 magic-192 W round (double-rounding flips), per-quarter y
stores for all tiles (HWDGE issue overhead).
"""

from contextlib import ExitStack

import numpy as np

import concourse.bass as bass
import concourse.tile as tile
from concourse import bacc, bass_isa, mybir
from concourse.bass import ds, ts
from concourse.bass_utils import run_bass_kernel_spmd

F32 = mybir.dt.float32
BF16 = mybir.dt.bfloat16
FP8 = mybir.dt.float8e4
AF = mybir.ActivationFunctionType
OP = mybir.AluOpType
AX = mybir.AxisListType

B, S, D_IN, D_OUT = 4, 4096, 2048, 2048
N_CORES = 8
TOK = B * S                # 16384 tokens
TPC = TOK // N_CORES       # 2048 tokens per core
NT = TPC // 128            # 16 token tiles per core
NB = D_IN // 128           # 16 contraction (k) blocks
NO = D_OUT // 512          # 4 output column blocks
HALF = D_OUT // 2          # 1024
CM = 12582912.0            # 1.5 * 2^23: fp32 RNE rounding magic
QMAX = 127.0

KNOBS = {
    "ldx_bufs": 2,
    "xq_bufs": 1,
    "t1_bufs": 2,
    "xqt_bufs": 3,
    "ys_bufs": 2,
    "psum_bufs": 8,
    "clamp_engine": "gpsimd",
}

_CACHE = {}


def _emit(tc: tile.TileContext, x_d: bass.AP, w_d: bass.AP, y_d: bass.AP):
    nc = tc.nc
    clamp_eng = {"gpsimd": nc.gpsimd, "vector": nc.vector}[KNOBS["clamp_engine"]]
    with ExitStack() as ctx:
        wres = ctx.enter_context(tc.tile_pool(name="wres", bufs=1))
        wqp = ctx.enter_context(tc.tile_pool(name="wqp", bufs=1))
        ldx = ctx.enter_context(tc.tile_pool(name="ldx", bufs=KNOBS["ldx_bufs"]))
        xqp = ctx.enter_context(tc.tile_pool(name="xqp", bufs=KNOBS["xq_bufs"]))
        xqtp = ctx.enter_context(tc.tile_pool(name="xqtp", bufs=KNOBS["xqt_bufs"]))
        ysp = ctx.enter_context(tc.tile_pool(name="ysp", bufs=KNOBS["ys_bufs"]))
        t1p = ctx.enter_context(tc.tile_pool(name="t1p", bufs=KNOBS["t1_bufs"]))
        stats = ctx.enter_context(tc.tile_pool(name="stats", bufs=4))
        consts = ctx.enter_context(tc.tile_pool(name="consts", bufs=1))
        psum = ctx.enter_context(
            tc.tile_pool(name="psum", bufs=KNOBS["psum_bufs"], space=bass.MemorySpace.PSUM)
        )

        # ---- issue the ENTIRE W stream first: 16 x 1MB, all resident,
        # no pool recycling -> the sync ring drains at HBM line rate.
        wt = []
        for j in range(NB):
            t = wres.tile([128, D_OUT], F32, tag=f"w{j}", name=f"w{j}")
            nc.sync.dma_start(t, w_d[ts(j, 128), :])
            wt.append(t)

        xtiles = {}

        def x_load(t):
            xt = ldx.tile([128, D_IN], BF16, tag="ldx", name=f"x{t}")
            nc.gpsimd.dma_start(xt, x_d[ts(t, 128), :])  # f32 -> bf16 cast DMA
            xtiles[t] = xt

        x_load(0)
        x_load(1)

        cpos = consts.tile([128, 1], F32, tag="cpos")
        nc.vector.memset(cpos, CM)
        czero = consts.tile([128, 1], F32, tag="czero")
        nc.vector.memset(czero, 0.0)
        # dummy activation on a ready constant: triggers the one-time
        # ACT_TABLE_LOAD during DMA warmup instead of on the critical chain
        warm = stats.tile([128, 1], F32, tag="warm")
        nc.scalar.activation(warm, czero, AF.Abs, bias=czero)

        # pass-1 abs-sums: even j on DVE (reduce, non-clobbering), odd j on
        # ACT (Abs -> throwaway bf16 scratch in the idle ys ring, accum_out
        # catches the column sum) so neither engine gates the W stream.
        wsumsD = stats.tile([128, NB // 2], F32, tag="wsumsD")
        wsumsA = stats.tile([128, NB // 2], F32, tag="wsumsA")

        def pass1(j):
            if j % 2 == 0:
                nc.vector.reduce_sum(
                    wsumsD[:, ds(j // 2, 1)], wt[j], axis=AX.X,
                    apply_absolute_value=True,
                )
            else:
                scr = ysp.tile([128, D_OUT], BF16, tag="ys", name=f"p1scr{j}")
                nc.scalar.activation(
                    scr, wt[j], AF.Abs, bias=czero,
                    accum_out=wsumsA[:, ds(j // 2, 1)],
                )

        xscales = {}

        def x_stats(t):
            a = stats.tile([128, 1], F32, tag="xa", name=f"xa{t}")
            nc.vector.reduce_max(a, xtiles[t], axis=AX.X, apply_absolute_value=True)
            nc.vector.tensor_scalar(a, a, 1e-8, None, OP.max)
            r0 = stats.tile([128, 1], F32, tag="xr0", name=f"xr0{t}")
            nc.vector.reciprocal(r0, a)
            ntt = stats.tile([128, 1], F32, tag="xntt", name=f"xntt{t}")
            nc.vector.tensor_mul(ntt, a, r0)
            nc.vector.tensor_scalar(ntt, ntt, -1.0, 2.0, OP.mult, OP.add)
            s = stats.tile([128, 1], F32, tag="xs", name=f"xs{t}")
            nc.vector.tensor_mul(s, r0, ntt)
            nc.vector.tensor_scalar(s, s, QMAX, None, OP.mult)  # 127/a
            xscales[t] = (a, s)

        xqts = {}

        def x_quant(t):
            a, s = xscales[t]
            xt = xtiles.pop(t)
            xq = xqp.tile([128, D_IN], BF16, tag="xq", name=f"xq{t}")
            for h in range(2):
                t1 = t1p.tile([128, HALF], F32, tag="t1", name=f"xt1_{t}_{h}")
                nc.scalar.activation(
                    t1, xt[:, ds(h * HALF, HALF)], AF.Identity, bias=cpos, scale=s
                )
                nc.vector.tensor_scalar(
                    xq[:, ds(h * HALF, HALF)], t1, -CM, None, OP.add
                )
            xqT = xqtp.tile([128, NB, 128], BF16, tag="xqT", name=f"xqT{t}")
            nc.sync.dma_start(xqT, xq, transpose=True)
            xqts[t] = xqT

        souts = {}

        def x_sout(t):
            a, _ = xscales[t]
            so = stats.tile([128, 1], F32, tag="xso", name=f"xso{t}")
            nc.vector.tensor_scalar(so, a, ws127, None, OP.mult)
            souts[t] = so

        # interleave pass-1 with x prep for the first two tiles
        pass1(0)
        pass1(1)
        pass1(2)
        x_stats(0)
        x_quant(0)
        pass1(3)
        pass1(4)
        pass1(5)
        x_stats(1)
        x_quant(1)
        for j in range(6, NB):
            pass1(j)

        # ---- w_scale ----
        wsD = stats.tile([128, 1], F32, tag="wsD")
        nc.vector.reduce_sum(wsD, wsumsD, axis=AX.X)
        wsA = stats.tile([128, 1], F32, tag="wsA")
        nc.vector.reduce_sum(wsA, wsumsA, axis=AX.X)
        wsum_p = stats.tile([128, 1], F32, tag="wsp")
        nc.vector.tensor_add(wsum_p, wsD, wsA)
        wsum_all = stats.tile([128, 1], F32, tag="wsa")
        nc.gpsimd.partition_all_reduce(wsum_all, wsum_p, 128, bass_isa.ReduceOp.add)
        wscale = consts.tile([128, 1], F32, tag="wscale")
        nc.vector.tensor_scalar(
            wscale, wsum_all, 1.0 / (D_OUT * D_IN), 1e-6, OP.mult, OP.max
        )
        r0 = stats.tile([128, 1], F32, tag="wr0")
        nc.vector.reciprocal(r0, wscale)
        ntt = stats.tile([128, 1], F32, tag="wntt")
        nc.vector.tensor_mul(ntt, wscale, r0)
        nc.vector.tensor_scalar(ntt, ntt, -1.0, 2.0, OP.mult, OP.add)
        rws = consts.tile([128, 1], F32, tag="rws")
        nc.vector.tensor_mul(rws, r0, ntt)
        ws127 = consts.tile([128, 1], F32, tag="ws127")
        nc.vector.tensor_scalar(ws127, wscale, 1.0 / QMAX, None, OP.mult)
        x_sout(0)
        x_sout(1)

        # x tiles 2,3 load during the chase (gpsimd FIFO holds them behind
        # the all-reduce so they stay off the W stream), prep runs post-chase
        x_load(2)
        x_load(3)
        x_load(4)

        # ---- W quantize stream + PE chase-ramp ----
        wq = [
            wqp.tile([128, D_OUT], FP8, tag=f"wq{b}", name=f"wq{b}")
            for b in range(NB)
        ]

        def w_quant(b):
            for h in range(2):
                t1 = t1p.tile([128, HALF], F32, tag="t1", name=f"wt1_{b}_{h}")
                nc.scalar.activation(
                    t1, wt[b][:, ds(h * HALF, HALF)], AF.Identity,
                    bias=cpos, scale=rws,
                )
                nc.vector.tensor_scalar(
                    wq[b][:, ds(h * HALF, HALF)], t1, -CM, None, OP.add
                )
            clamp_eng.tensor_scalar(wq[b], wq[b], -1.0, 1.0, OP.max, OP.min)

        for b in range(NB):
            w_quant(b)

        # 8 PSUM cells (tiles 0-1 x col-blocks 0-3) accumulate each b as its
        # wq lands; PE consumes at ~1.7us/b vs ~2.2us/b quantize rate.
        chase = [(t, no) for t in range(2) for no in range(NO)]
        pss = {}
        for c, (t, no) in enumerate(chase):
            pss[c] = psum.tile([128, 512], F32, tag="ps", name=f"cps{c}")
        for b in range(NB):
            for c, (t, no) in enumerate(chase):
                nc.tensor.matmul(
                    pss[c],
                    xqts[t][:, b, :],
                    wq[b][:, ds(no * 512, 512)],
                    start=(b == 0),
                    stop=(b == NB - 1),
                )

        # x prep for tiles 2,3 (ACT/DVE free again after the wq stream)
        x_stats(2)
        x_quant(2)
        x_sout(2)
        x_stats(3)
        x_quant(3)
        x_sout(3)

        ys = {}

        def y_tile(t):
            if t not in ys:
                ys[t] = ysp.tile([128, D_OUT], BF16, tag="ys", name=f"ys{t}")
            return ys[t]

        for c, (t, no) in enumerate(chase):
            nc.vector.tensor_scalar(
                y_tile(t)[:, ds(no * 512, 512)], pss[c], souts[t], None, OP.mult
            )
        del pss

        def y_store(t):
            nc.sync.dma_start(y_d[ts(t, 128), :], ys.pop(t))
            del xqts[t]

        y_store(0)
        y_store(1)

        # ---- steady state: x-prep two tiles ahead ----
        def cell(no, t, store_quarter=False):
            ps = psum.tile([128, 512], F32, tag="ps")
            xqT = xqts[t]
            for b in range(NB):
                nc.tensor.matmul(
                    ps,
                    xqT[:, b, :],
                    wq[b][:, ds(no * 512, 512)],
                    start=(b == 0),
                    stop=(b == NB - 1),
                )
            nc.vector.tensor_scalar(
                y_tile(t)[:, ds(no * 512, 512)], ps, souts[t], None, OP.mult
            )
            if store_quarter:
                nc.sync.dma_start(
                    y_d[ts(t, 128), ds(no * 512, 512)],
                    ys[t][:, ds(no * 512, 512)],
                )

        for t in range(2, NT):
            last = t == NT - 1
            if t + 3 < NT:
                x_load(t + 3)
            if t + 2 < NT:
                x_stats(t + 2)
                x_quant(t + 2)
                x_sout(t + 2)
            for no in range(NO):
                cell(no, t, store_quarter=last)
            if last:
                ys.pop(t)
                del xqts[t]
            else:
                y_store(t)


def _build():
    key = tuple(sorted((k, str(v)) for k, v in KNOBS.items()))
    if key in _CACHE:
        return _CACHE[key]
    nc = bacc.Bacc(
        "TRN2", target_bir_lowering=False, debug=False, num_devices=N_CORES
    )
    x_d = nc.dram_tensor("x", [TPC, D_IN], F32, kind="ExternalInput").ap()
    # w is fed TRANSPOSED by the host: [k, out]
    w_d = nc.dram_tensor("w", [D_IN, D_OUT], F32, kind="ExternalInput").ap()
    y_d = nc.dram_tensor("y", [TPC, D_OUT], BF16, kind="ExternalOutput").ap()
    with tile.TileContext(nc) as tc:
        _emit(tc, x_d, w_d, y_d)
    nc.compile()
    _CACHE[key] = nc
    return nc


_last_result = None  # BassKernelResults of the most recent run (for profiling)


def kernel(x: np.ndarray, weight: np.ndarray, trace: bool = False) -> np.ndarray:
    global _last_result
    nc = _build()
    xf = np.ascontiguousarray(x.reshape(TOK, D_IN), dtype=np.float32)
    wT = np.ascontiguousarray(weight.T, dtype=np.float32)
    in_maps = [
        {"x": xf[c * TPC:(c + 1) * TPC], "w": wT}
        for c in range(N_CORES)
    ]
    res = run_bass_kernel_spmd(nc, in_maps, list(range(N_CORES)), trace=trace)
    _last_result = res
    y = np.concatenate(
        [np.asarray(res.results[c]["y"]) for c in range(N_CORES)], axis=0
    )
    return y.reshape(B, S, D_OUT).astype(np.float32)


# revision 4
# speedup vs baseline: 2.3000x; 2.3000x over previous
"""BitLinear (ternary-weight + 8-bit-activation quantized matmul) on 8 TRN2 cores.

Strategy: data-parallel over tokens. Each core gets 2048 of the 16384 tokens
plus the full weight matrix, computes the whole BitLinear forward for its
token shard on device, and the host concatenates the shards.

Math (must match the jax reference):
  w_scale = max(mean(|W|), 1e-6)                       (scalar)
  w_q     = clip(round(W / w_scale), -1, 1)            (ternary)
  a       = clip(max_i |x|, 1e-8, inf)                 (per token)
  x_q     = clip(round(x * 127 / a), -127, 127)        (8-bit ints)
  y       = (x_q @ w_q^T) * w_scale * a / 127

v3 design (roofline: 221us bf16 GEMM + ~50us W stream; v1 394 / v2 340):
  - w_scale is extremely sensitive (2e-4 rel deviation flips ternary weights
    near .5 boundaries -> 3e-2 err), so pass 1 must abs-sum the full fp32 W.
    W is fed TRANSPOSED (wT[k,out]) and is FULLY RESIDENT in fp32 (128KB of
    ~207KB/partition SBUF): zero re-read, W streams exactly once as 16 x 1MB
    DMAs all issued upfront with no pool waits -> HBM line rate.
  - pass-1 abs-sums alternate DVE reduce_sum (non-clobbering, even j) and
    ACT Abs->bf16 scratch with accum_out (odd j; bf16-rounded |W| sums are
    random-error ~1e-6 rel over 4.2M elements, harmless) so neither engine
    serializes the stream; w_scale lands ~2.5us after the last W tile.
  - the x side runs in BF16: x is cast f32->bf16 during the DMA itself
    (SWDGE/gpsimd ring, separate from the W stream's HWDGE ring). a_scale
    and x_q derive from the same bf16 x, so a cancels between quantize and
    rescale; measured 7.0e-3 total err (budget 2e-2). x_q flips from bf16
    x are per-token-random and wash out, unlike W-side flips.
  - rounding is the fp32 magic-number trick (+1.5*2^23, exact RNE; a bf16
    magic of +192 double-rounds and flips ~16 weights at the .5 boundary =
    up to ~1.8e-2 err, rejected). Quantize runs on [128,1024] halves
    through a shared 2-buf t1 pool. W: ACT magic -> DVE subtract into fp8
    (ints <= 8 exact in e4m3) -> full-row in-place fp8 clamp on GPSIMD
    (keeps DVE off the chase critical path). x: ACT magic -> DVE subtract
    to bf16, no clamp needed (|x*127/a| <= 127 by construction).
  - GEMM ramp: 8 PSUM cells (tiles 0-1 x 4 col-blocks) accumulate each b
    the moment its wq lands (PE chases the ACT-bound ~2.2us/b quantize
    stream at 1.7us/b), so the PE starts ~4us after w_scale.
  - steady state: x-prep pipelined two tiles ahead (xqT ring of 3), loads
    three ahead; per-iter budget PE 13.8us vs DVE ~7.2, ACT ~2.2.
  - y is stored bf16 (host upcasts); the last tile stores per-quarter to
    shorten the drain tail.
Dead ends (measured): sampled/bf16 w_scale (1.7-4e-2 err), sharded pass-1 +
AllReduce (~80us collective), fp8 DoubleRow x_q (exact hi/lo needs 2x
virtual MACs = breakeven minus overheads; single-pass fp8 approx 2.3e-2 >
budget), bf16 magic-192 W round (double-rounding flips), per-quarter y
stores for all tiles (HWDGE issue overhead).
"""

from contextlib import ExitStack

import numpy as np

import concourse.bass as bass
import concourse.tile as tile
from concourse import bacc, bass_isa, mybir
from concourse.bass import ds, ts
from concourse.bass_utils import run_bass_kernel_spmd

F32 = mybir.dt.float32
BF16 = mybir.dt.bfloat16
FP8 = mybir.dt.float8e4
AF = mybir.ActivationFunctionType
OP = mybir.AluOpType
AX = mybir.AxisListType

B, S, D_IN, D_OUT = 4, 4096, 2048, 2048
N_CORES = 8
TOK = B * S                # 16384 tokens
TPC = TOK // N_CORES       # 2048 tokens per core
NT = TPC // 128            # 16 token tiles per core
NB = D_IN // 128           # 16 contraction (k) blocks
NO = D_OUT // 512          # 4 output column blocks
HALF = D_OUT // 2          # 1024
CM = 12582912.0            # 1.5 * 2^23: fp32 RNE rounding magic
QMAX = 127.0

KNOBS = {
    "ldx_bufs": 2,
    "xq_bufs": 1,
    "t1_bufs": 2,
    "xqt_bufs": 3,
    "ys_bufs": 2,
    "psum_bufs": 8,
    "clamp_engine": "vector",
}

_CACHE = {}


def _emit(tc: tile.TileContext, x_d: bass.AP, w_d: bass.AP, y_d: bass.AP):
    nc = tc.nc
    clamp_eng = {"gpsimd": nc.gpsimd, "vector": nc.vector}[KNOBS["clamp_engine"]]
    with ExitStack() as ctx:
        wres = ctx.enter_context(tc.tile_pool(name="wres", bufs=1))
        wqp = ctx.enter_context(tc.tile_pool(name="wqp", bufs=1))
        ldx = ctx.enter_context(tc.tile_pool(name="ldx", bufs=KNOBS["ldx_bufs"]))
        xqp = ctx.enter_context(tc.tile_pool(name="xqp", bufs=KNOBS["xq_bufs"]))
        xqtp = ctx.enter_context(tc.tile_pool(name="xqtp", bufs=KNOBS["xqt_bufs"]))
        ysp = ctx.enter_context(tc.tile_pool(name="ysp", bufs=KNOBS["ys_bufs"]))
        t1p = ctx.enter_context(tc.tile_pool(name="t1p", bufs=KNOBS["t1_bufs"]))
        stats = ctx.enter_context(tc.tile_pool(name="stats", bufs=4))
        consts = ctx.enter_context(tc.tile_pool(name="consts", bufs=1))
        psum = ctx.enter_context(
            tc.tile_pool(name="psum", bufs=KNOBS["psum_bufs"], space=bass.MemorySpace.PSUM)
        )

        # ---- issue the ENTIRE W stream first: 16 x 1MB, all resident,
        # no pool recycling -> the sync ring drains at HBM line rate.
        wt = []
        for j in range(NB):
            t = wres.tile([128, D_OUT], F32, tag=f"w{j}", name=f"w{j}")
            nc.sync.dma_start(t, w_d[ts(j, 128), :])
            wt.append(t)

        xtiles = {}

        def x_load(t):
            xt = ldx.tile([128, D_IN], BF16, tag="ldx", name=f"x{t}")
            nc.gpsimd.dma_start(xt, x_d[ts(t, 128), :])  # f32 -> bf16 cast DMA
            xtiles[t] = xt

        x_load(0)
        x_load(1)

        cpos = consts.tile([128, 1], F32, tag="cpos")
        nc.vector.memset(cpos, CM)
        czero = consts.tile([128, 1], F32, tag="czero")
        nc.vector.memset(czero, 0.0)
        # dummy activation on a ready constant: triggers the one-time
        # ACT_TABLE_LOAD during DMA warmup instead of on the critical chain
        warm = stats.tile([128, 1], F32, tag="warm")
        nc.scalar.activation(warm, czero, AF.Abs, bias=czero)

        # pass-1 abs-sums: even j on DVE (reduce, non-clobbering), odd j on
        # ACT (Abs -> throwaway bf16 scratch in the idle ys ring, accum_out
        # catches the column sum) so neither engine gates the W stream.
        wsumsD = stats.tile([128, NB // 2], F32, tag="wsumsD")
        wsumsA = stats.tile([128, NB // 2], F32, tag="wsumsA")

        def pass1(j):
            if j % 2 == 0:
                nc.vector.reduce_sum(
                    wsumsD[:, ds(j // 2, 1)], wt[j], axis=AX.X,
                    apply_absolute_value=True,
                )
            else:
                scr = ysp.tile([128, D_OUT], BF16, tag="ys", name=f"p1scr{j}")
                nc.scalar.activation(
                    scr, wt[j], AF.Abs, bias=czero,
                    accum_out=wsumsA[:, ds(j // 2, 1)],
                )

        xscales = {}

        def x_stats(t):
            a = stats.tile([128, 1], F32, tag="xa", name=f"xa{t}")
            nc.vector.reduce_max(a, xtiles[t], axis=AX.X, apply_absolute_value=True)
            nc.vector.tensor_scalar(a, a, 1e-8, None, OP.max)
            r0 = stats.tile([128, 1], F32, tag="xr0", name=f"xr0{t}")
            nc.vector.reciprocal(r0, a)
            ntt = stats.tile([128, 1], F32, tag="xntt", name=f"xntt{t}")
            nc.vector.tensor_mul(ntt, a, r0)
            nc.vector.tensor_scalar(ntt, ntt, -1.0, 2.0, OP.mult, OP.add)
            s = stats.tile([128, 1], F32, tag="xs", name=f"xs{t}")
            nc.vector.tensor_mul(s, r0, ntt)
            nc.vector.tensor_scalar(s, s, QMAX, None, OP.mult)  # 127/a
            xscales[t] = (a, s)

        xqts = {}

        def x_quant(t):
            a, s = xscales[t]
            xt = xtiles.pop(t)
            xq = xqp.tile([128, D_IN], BF16, tag="xq", name=f"xq{t}")
            for h in range(2):
                t1 = t1p.tile([128, HALF], F32, tag="t1", name=f"xt1_{t}_{h}")
                nc.scalar.activation(
                    t1, xt[:, ds(h * HALF, HALF)], AF.Identity, bias=cpos, scale=s
                )
                nc.vector.tensor_scalar(
                    xq[:, ds(h * HALF, HALF)], t1, -CM, None, OP.add
                )
            xqT = xqtp.tile([128, NB, 128], BF16, tag="xqT", name=f"xqT{t}")
            nc.sync.dma_start(xqT, xq, transpose=True)
            xqts[t] = xqT

        souts = {}

        def x_sout(t):
            a, _ = xscales[t]
            so = stats.tile([128, 1], F32, tag="xso", name=f"xso{t}")
            nc.vector.tensor_scalar(so, a, ws127, None, OP.mult)
            souts[t] = so

        # interleave pass-1 with x prep for the first two tiles
        pass1(0)
        pass1(1)
        pass1(2)
        x_stats(0)
        x_quant(0)
        pass1(3)
        pass1(4)
        pass1(5)
        x_stats(1)
        x_quant(1)
        for j in range(6, NB):
            pass1(j)

        # ---- w_scale ----
        wsD = stats.tile([128, 1], F32, tag="wsD")
        nc.vector.reduce_sum(wsD, wsumsD, axis=AX.X)
        wsA = stats.tile([128, 1], F32, tag="wsA")
        nc.vector.reduce_sum(wsA, wsumsA, axis=AX.X)
        wsum_p = stats.tile([128, 1], F32, tag="wsp")
        nc.vector.tensor_add(wsum_p, wsD, wsA)
        wsum_all = stats.tile([128, 1], F32, tag="wsa")
        nc.gpsimd.partition_all_reduce(wsum_all, wsum_p, 128, bass_isa.ReduceOp.add)
        wscale = consts.tile([128, 1], F32, tag="wscale")
        nc.vector.tensor_scalar(
            wscale, wsum_all, 1.0 / (D_OUT * D_IN), 1e-6, OP.mult, OP.max
        )
        r0 = stats.tile([128, 1], F32, tag="wr0")
        nc.vector.reciprocal(r0, wscale)
        ntt = stats.tile([128, 1], F32, tag="wntt")
        nc.vector.tensor_mul(ntt, wscale, r0)
        nc.vector.tensor_scalar(ntt, ntt, -1.0, 2.0, OP.mult, OP.add)
        rws = consts.tile([128, 1], F32, tag="rws")
        nc.vector.tensor_mul(rws, r0, ntt)
        ws127 = consts.tile([128, 1], F32, tag="ws127")
        nc.vector.tensor_scalar(ws127, wscale, 1.0 / QMAX, None, OP.mult)
        x_sout(0)
        x_sout(1)

        # x tiles 2,3 load during the chase (gpsimd FIFO holds them behind
        # the all-reduce so they stay off the W stream), prep runs post-chase
        x_load(2)
        x_load(3)
        x_load(4)

        # ---- W quantize stream + PE chase-ramp ----
        wq = [
            wqp.tile([128, D_OUT], FP8, tag=f"wq{b}", name=f"wq{b}")
            for b in range(NB)
        ]

        def w_quant(b):
            for h in range(2):
                t1 = t1p.tile([128, HALF], F32, tag="t1", name=f"wt1_{b}_{h}")
                nc.scalar.activation(
                    t1, wt[b][:, ds(h * HALF, HALF)], AF.Identity,
                    bias=cpos, scale=rws,
                )
                nc.vector.tensor_scalar(
                    wq[b][:, ds(h * HALF, HALF)], t1, -CM, None, OP.add
                )
            clamp_eng.tensor_scalar(wq[b], wq[b], -1.0, 1.0, OP.max, OP.min)

        for b in range(NB):
            w_quant(b)

        # 8 PSUM cells (tiles 0-1 x col-blocks 0-3) accumulate each b as its
        # wq lands; PE consumes at ~1.7us/b vs ~2.2us/b quantize rate.
        chase = [(t, no) for t in range(2) for no in range(NO)]
        pss = {}
        for c, (t, no) in enumerate(chase):
            pss[c] = psum.tile([128, 512], F32, tag="ps", name=f"cps{c}")
        for b in range(NB):
            for c, (t, no) in enumerate(chase):
                nc.tensor.matmul(
                    pss[c],
                    xqts[t][:, b, :],
                    wq[b][:, ds(no * 512, 512)],
                    start=(b == 0),
                    stop=(b == NB - 1),
                )

        # x prep for tiles 2,3 (ACT/DVE free again after the wq stream)
        x_stats(2)
        x_quant(2)
        x_sout(2)
        x_stats(3)
        x_quant(3)
        x_sout(3)

        ys = {}

        def y_tile(t):
            if t not in ys:
                ys[t] = ysp.tile([128, D_OUT], BF16, tag="ys", name=f"ys{t}")
            return ys[t]

        for c, (t, no) in enumerate(chase):
            nc.vector.tensor_scalar(
                y_tile(t)[:, ds(no * 512, 512)], pss[c], souts[t], None, OP.mult
            )
        del pss

        def y_store(t):
            nc.sync.dma_start(y_d[ts(t, 128), :], ys.pop(t))
            del xqts[t]

        y_store(0)
        y_store(1)

        # ---- steady state: x-prep two tiles ahead ----
        def cell(no, t, store_quarter=False):
            ps = psum.tile([128, 512], F32, tag="ps")
            xqT = xqts[t]
            for b in range(NB):
                nc.tensor.matmul(
                    ps,
                    xqT[:, b, :],
                    wq[b][:, ds(no * 512, 512)],
                    start=(b == 0),
                    stop=(b == NB - 1),
                )
            nc.vector.tensor_scalar(
                y_tile(t)[:, ds(no * 512, 512)], ps, souts[t], None, OP.mult
            )
            if store_quarter:
                nc.sync.dma_start(
                    y_d[ts(t, 128), ds(no * 512, 512)],
                    ys[t][:, ds(no * 512, 512)],
                )

        for t in range(2, NT):
            last = t == NT - 1
            if t + 3 < NT:
                x_load(t + 3)
            if t + 2 < NT:
                x_stats(t + 2)
                x_quant(t + 2)
                x_sout(t + 2)
            for no in range(NO):
                cell(no, t, store_quarter=last)
            if last:
                ys.pop(t)
                del xqts[t]
            else:
                y_store(t)


def _build():
    key = tuple(sorted((k, str(v)) for k, v in KNOBS.items()))
    if key in _CACHE:
        return _CACHE[key]
    nc = bacc.Bacc(
        "TRN2", target_bir_lowering=False, debug=False, num_devices=N_CORES
    )
    x_d = nc.dram_tensor("x", [TPC, D_IN], F32, kind="ExternalInput").ap()
    # w is fed TRANSPOSED by the host: [k, out]
    w_d = nc.dram_tensor("w", [D_IN, D_OUT], F32, kind="ExternalInput").ap()
    y_d = nc.dram_tensor("y", [TPC, D_OUT], BF16, kind="ExternalOutput").ap()
    with tile.TileContext(nc) as tc:
        _emit(tc, x_d, w_d, y_d)
    nc.compile()
    _CACHE[key] = nc
    return nc


_last_result = None  # BassKernelResults of the most recent run (for profiling)


def kernel(x: np.ndarray, weight: np.ndarray, trace: bool = False) -> np.ndarray:
    global _last_result
    nc = _build()
    xf = np.ascontiguousarray(x.reshape(TOK, D_IN), dtype=np.float32)
    wT = np.ascontiguousarray(weight.T, dtype=np.float32)
    in_maps = [
        {"x": xf[c * TPC:(c + 1) * TPC], "w": wT}
        for c in range(N_CORES)
    ]
    res = run_bass_kernel_spmd(nc, in_maps, list(range(N_CORES)), trace=trace)
    _last_result = res
    y = np.concatenate(
        [np.asarray(res.results[c]["y"]) for c in range(N_CORES)], axis=0
    )
    return y.reshape(B, S, D_OUT).astype(np.float32)
